# revision 4
# baseline (speedup 1.0000x reference)
"""Trainium2 Bass kernel for nn_BoundaryLoss2 (dice + BCE + boundary loss).

Strategy (data-parallel over batch, one sample per core, 8 cores):
  The expensive part is the exact euclidean distance transform (EDT) of the
  target mask (and its complement) per sample.  The reference computes
      d2[i,j] = min_j' ( g[i,j']^2 + (j-j')^2 ),   g = vertical L1 distance
  For 50%-density random masks the true distances are tiny (max d2 == 9 for
  the actual inputs), so a windowed min-plus with radius K is *exact* as long
  as max(d2) <= K^2; this condition is verified on-device (max-reduce of the
  computed field) and a host-side numpy fallback guarantees correctness
  otherwise.

  Per core pipeline:
    - load logits/targets (256x256) in row-layout [128 part, 2 group, 256 col]
    - build mask cost maps c in {0, BIG} (bf16), PE-transpose to column-layout
    - vertical L1 distance with two tensor_tensor_scan passes (fwd + reversed)
    - PE-transpose back, squaring fused into the PSUM->SBUF copy (ScalarE)
    - windowed parabola pass: acc = min_d ( g2(col+d) + d^2 ), |d| <= K,
      via tensor_tensor(min) + fused scalar_tensor_tensor(add, min)
    - d1 - d0 is the signed distance term (d1 zero on positives, d0 zero on
      negatives), boundary sum = sum(sig*(d1-d0)) + sum(sig*t)
    - all loss terms reduce to per-partition partial sums -> [128, 8] output
  Host gathers the 8 [128,8] stats tensors and combines the scalars.
"""

import numpy as np

import concourse.bacc as bacc
import concourse.bass as bass
import concourse.tile as tile
from concourse import mybir
from concourse.bass_utils import run_bass_kernel_spmd
from concourse.masks import make_identity

P = 128
H = 256
W = 256
NCORES = 8
B = 8
K = 4  # window radius; result exact iff max(d2) <= K*K (checked on device)
BIG = 30000.0
GAP = 8  # border gap in the parabola tile (>= K, 8 keeps alignment)
SMOOTH = 1e-5
F32 = mybir.dt.float32
BF16 = mybir.dt.bfloat16

# stats column layout
S_SIG, S_T, S_LT, S_ST, S_SP, S_SDQ, S_MAXW2, S_PAD = range(8)


def build_boundary_loss_core(tc, stats_out, logits_in, targets_in):
    """Emit the per-core kernel. APs: stats_out [P,8] f32 (DRAM out),
    logits_in/targets_in [H,W] f32 (DRAM in)."""
    nc = tc.nc
    Alu = mybir.AluOpType
    Act = mybir.ActivationFunctionType
    WP = W + 2 * GAP  # padded parabola row width
    WR = WP - 8  # parabola op region width (reads stay in bounds for |d|<=4)

    with (
        tc.tile_pool(name="consts", bufs=1) as consts,
        tc.tile_pool(name="work", bufs=1) as work,
        tc.tile_pool(name="psum", bufs=4, space=bass.MemorySpace.PSUM) as psum,
    ):
        ident = consts.tile([P, P], BF16)
        make_identity(nc, ident)
        ones_h = consts.tile([P, H], BF16)
        nc.gpsimd.memset(ones_h, 1.0)
        bias0 = consts.tile([P, 1], F32)
        nc.gpsimd.memset(bias0, 0.0)
        bias1 = consts.tile([P, 1], F32)
        nc.gpsimd.memset(bias1, 1.0)
        bias_big = consts.tile([P, 1], F32)
        nc.gpsimd.memset(bias_big, BIG)

        # ---- load inputs in row layout [p, g, j] = img[g*128+p, j] ----
        t_b = work.tile([P, 2, W], F32)
        nc.sync.dma_start(out=t_b, in_=targets_in.rearrange("(g p) w -> p g w", p=P))
        l_b = work.tile([P, 2, W], F32)
        nc.sync.dma_start(out=l_b, in_=logits_in.rearrange("(g p) w -> p g w", p=P))

        # ---- mask cost maps: c[m] = 0 where (t == m-target) else BIG ----
        # m=0: positives mask (t==1 -> 0), m=1: negatives mask (t==0 -> 0)
        c_b = work.tile([P, 2, 2, W], BF16)  # [p, m, g, j]
        nc.scalar.activation(c_b[:, 0], t_b, Act.Identity, bias=bias_big, scale=-BIG)
        nc.scalar.activation(c_b[:, 1], t_b, Act.Copy, scale=BIG)

        # ---- transpose to column layout [p, m, cg, i] = c[m][i, cg*128+p] ----
        cA = work.tile([P, 2, 2, H], BF16)
        for m in range(2):
            for g in range(2):
                for cg in range(2):
                    pt = psum.tile([P, P], BF16)
                    nc.tensor.transpose(pt, c_b[:, m, g, cg * P:(cg + 1) * P], ident)
                    nc.scalar.copy(cA[:, m, cg, g * P:(g + 1) * P], pt)

        # ---- vertical L1 distance: two scans (fwd, then bwd over fwd) ----
        ft = work.tile([P, 2, 2, H], BF16)
        gt = work.tile([P, 2, 2, H], BF16)
        for m in range(2):
            for cg in range(2):
                nc.vector.tensor_tensor_scan(
                    ft[:, m, cg], ones_h, cA[:, m, cg], BIG, Alu.add, Alu.min)
                nc.vector.tensor_tensor_scan(
                    gt[:, m, cg][:, ::-1], ones_h, ft[:, m, cg][:, ::-1],
                    BIG, Alu.add, Alu.min)

        # ---- transpose back to row layout, squaring fused into the copy ----
        g2b = work.tile([P, 2, 2, WP], BF16)  # [p, m, g, GAP+j]
        nc.gpsimd.memset(g2b, BIG)
        for m in range(2):
            for cg in range(2):
                for g in range(2):
                    pt = psum.tile([P, P], BF16)
                    nc.tensor.transpose(pt, gt[:, m, cg, g * P:(g + 1) * P], ident)
                    nc.scalar.activation(
                        g2b[:, m, g, GAP + cg * P:GAP + (cg + 1) * P], pt, Act.Square,
                        bias=bias0)

        # ---- windowed parabola pass along columns ----
        def sh(d):
            return g2b[:, :, :, 4 + d:4 + d + WR]

        u = work.tile([P, 2, 2, WR], BF16)
        acc = work.tile([P, 2, 2, WR], BF16)
        nc.vector.tensor_tensor(u, sh(-1), sh(1), Alu.min)
        nc.vector.scalar_tensor_tensor(acc, u, 1.0, sh(0), op0=Alu.add, op1=Alu.min)
        for d in range(2, K + 1):
            nc.vector.tensor_tensor(u, sh(-d), sh(d), Alu.min)
            nc.vector.scalar_tensor_tensor(
                acc, u, float(d * d), acc, op0=Alu.add, op1=Alu.min)

        # interior view: image col j lives at acc[..., 4 + j]
        w2 = acc[:, :, :, 4:4 + W]  # [p, m, g, j]

        stats = work.tile([P, 8], F32)
        nc.vector.memset(stats, 0.0)
        nc.vector.tensor_reduce(
            stats[:, S_MAXW2:S_MAXW2 + 1], w2, axis=mybir.AxisListType.XYZ,
            op=Alu.max)

        # ---- distances and loss terms ----
        dst = work.tile([P, 2, 2, W], F32)  # [p, m, g, j]; m=0 -> d1, m=1 -> d0
        nc.scalar.activation(dst, w2, Act.Sqrt, bias=bias0)

        sig = work.tile([P, 2, W], F32)
        nc.scalar.activation(
            sig, l_b, Act.Sigmoid, bias=bias0, accum_out=stats[:, S_SIG:S_SIG + 1])
        ex = work.tile([P, 2, W], F32)
        nc.scalar.activation(ex, l_b, Act.Exp, bias=bias0)
        sp = work.tile([P, 2, W], F32)  # softplus = ln(1 + exp(l))
        nc.scalar.activation(
            sp, ex, Act.Ln, bias=bias1, accum_out=stats[:, S_SP:S_SP + 1])

        tsum = work.tile([P, 2, W], F32)
        nc.vector.tensor_scalar(
            tsum, t_b, 1.0, None, op0=Alu.mult, op1=Alu.add,
            accum_out=stats[:, S_T:S_T + 1])

        lt = work.tile([P, 2, W], F32)
        nc.vector.scalar_tensor_tensor(
            lt, l_b, 1.0, t_b, op0=Alu.mult, op1=Alu.mult,
            accum_out=stats[:, S_LT:S_LT + 1])
        st = work.tile([P, 2, W], F32)
        nc.vector.scalar_tensor_tensor(
            st, sig, 1.0, t_b, op0=Alu.mult, op1=Alu.mult,
            accum_out=stats[:, S_ST:S_ST + 1])
        dq = work.tile([P, 2, W], F32)
        nc.vector.scalar_tensor_tensor(
            dq, dst[:, 0], 1.0, dst[:, 1], op0=Alu.mult, op1=Alu.subtract)
        sdq = work.tile([P, 2, W], F32)
        nc.vector.scalar_tensor_tensor(
            sdq, sig, 1.0, dq, op0=Alu.mult, op1=Alu.mult,
            accum_out=stats[:, S_SDQ:S_SDQ + 1])

        nc.sync.dma_start(out=stats_out, in_=stats)


_CACHE = {}


def _get_nc():
    if "nc" not in _CACHE:
        nc = bacc.Bacc("TRN2", target_bir_lowering=False, debug=False)
        logits_in = nc.dram_tensor("logits", (H, W), F32, kind="ExternalInput").ap()
        targets_in = nc.dram_tensor("targets", (H, W), F32, kind="ExternalInput").ap()
        stats_out = nc.dram_tensor("stats", (P, 8), F32, kind="ExternalOutput").ap()
        with tile.TileContext(nc) as tc:
            build_boundary_loss_core(tc, stats_out, logits_in, targets_in)
        nc.compile()
        _CACHE["nc"] = nc
    return _CACHE["nc"]


def combine_stats(stats):
    """stats: (NCORES, P, 8) -> scalar loss (np.float32). None if the
    windowed EDT was not provably exact (caller must fall back)."""
    maxw2 = float(stats[:, :, S_MAXW2].max())
    if maxw2 > K * K + 0.5:
        return None
    s = stats.sum(axis=1, dtype=np.float64)  # (NCORES, 8)
    n = float(B * H * W)
    s_sig, s_t = s[:, S_SIG], s[:, S_T]
    s_lt, s_st = s[:, S_LT], s[:, S_ST]
    s_sp, s_sdq = s[:, S_SP], s[:, S_SDQ]
    has_pos = s_t > 0
    inter = s_st.sum()
    union = s_sig.sum() + s_t.sum() + SMOOTH
    dice = 1.0 - (2.0 * inter + SMOOTH) / union
    bce = (s_sp.sum() - s_lt.sum()) / n
    bdy = np.where(has_pos, s_sdq + s_st, 0.0).sum() / n
    return np.float32(0.5 * dice + 0.5 * bce + 0.5 * bdy)


def run_device(logits, targets, trace=False, trace_cores=None):
    l = np.ascontiguousarray(np.asarray(logits, np.float32).reshape(NCORES, H, W))
    t = np.ascontiguousarray(np.asarray(targets, np.float32).reshape(NCORES, H, W))
    in_maps = [{"logits": l[i], "targets": t[i]} for i in range(NCORES)]
    nc = _get_nc()
    res = run_bass_kernel_spmd(
        nc, in_maps, core_ids=list(range(NCORES)), trace=trace,
        trace_cores=trace_cores)
    stats = np.stack([res.results[i]["stats"] for i in range(NCORES)])
    return stats, res


# ---------------- host fallback (exact reference semantics) ----------------

def _edt_np(mask):
    """Exact EDT (distance to nearest True) matching the reference."""
    h, w = mask.shape
    big = float(h * w)
    c = np.where(mask, 0.0, np.inf)
    f = np.empty((h, w))
    s = np.full((w,), big)
    for i in range(h):
        s = np.minimum(s + 1.0, c[i])
        f[i] = s
    g = np.empty((h, w))
    s = np.full((w,), big)
    for i in reversed(range(h)):
        s = np.minimum(s + 1.0, f[i])
        g[i] = s
    g2 = g * g
    jj = np.arange(w, dtype=np.float64)
    dj2 = (jj[:, None] - jj[None, :]) ** 2  # (j_out, j_src)
    d2 = np.empty((h, w))
    for i in range(h):
        d2[i] = (g2[i][None, :] + dj2).min(axis=1)
    return np.sqrt(d2)


def _fallback_loss(logits, targets):
    l = np.asarray(logits, np.float64).reshape(B, H, W)
    t = np.asarray(targets, np.float64).reshape(B, H, W)
    sig = 1.0 / (1.0 + np.exp(-l))
    inter = (sig * t).sum()
    union = sig.sum() + t.sum() + SMOOTH
    dice = 1.0 - (2.0 * inter + SMOOTH) / union
    bce = (np.logaddexp(l, 0.0) - l * t).mean()
    bdy_sum = 0.0
    for b_i in range(B):
        m = t[b_i] > 0.5
        if not m.any():
            continue
        d1 = _edt_np(m)
        d0 = _edt_np(~m)
        res = d1 * (1.0 - t[b_i]) - (d0 - 1.0) * t[b_i]
        bdy_sum += (sig[b_i] * res).sum()
    bdy = bdy_sum / float(B * H * W)
    return np.float32(0.5 * dice + 0.5 * bce + 0.5 * bdy)


def kernel(logits, targets):
    stats, _ = run_device(logits, targets)
    loss = combine_stats(stats)
    if loss is None:
        loss = _fallback_loss(logits, targets)
    return np.array(loss, dtype=np.float32)
